# revision 28
# baseline (speedup 1.0000x reference)
"""Trainium2 Bass kernel for nn_CIN: 3-layer Compressed Interaction Network.

Reference computation (per layer l, with x0 = x):
    z = einsum('bhd,bmd,ohm->bod', h, x0, W_l.reshape(o, h, m)) + b_l
    h = relu(z)
Output: concat over layers of sum_d(h)  -> (B, 384)

Strategy: data-parallel over batch across 8 cores (128 b/core). Per core,
batch is processed in 16 groups of 8 (free dim f = (b_l, d) = 512).
Each layer is a chain of matmul accumulations over chunks c of 128
(h, m)-pairs:
    P_c[p, f] = A_c[p, f] * B_c[p, f]               (DVE fp16 multiply, 2x)
    z[o, f]  += W_c^T @ P_c                         (PE fp16, fp32 PSUM)
Layers 1-2 chunk as (h all 128, m = c): A_c = h (reused via a stride-0 AP),
B_c = x0 row m broadcast to 128 partitions by HWDGE DMA (the x0 broadcast
is shared by both layers). Layer 0 exploits the x (x) x symmetry: only the
528 pairs (a <= b) are materialized (5 chunks, host-symmetrized W0 with
off-diagonal weights doubled); both operands come pair-gathered from HBM.
ReLU runs on ACT; the d-sums come from ACT Copy+accum_out ops off the
critical path. Groups are processed in interleaved pairs to hide the
serial multiply -> matmul -> relu chain at layer boundaries.

Measured on 8 axon-tunneled trn2 cores: ~330 us end-to-end, rel err
~5e-4 vs the fp32 reference (fp16 operands, fp32 PSUM accumulation).
"""

import os
import sys

import numpy as np

for _p in ("/opt/trn_rl_repo", "/root/.axon_site/_ro/trn_rl_repo"):
    if os.path.isdir(_p) and _p not in sys.path:
        sys.path.append(_p)

import concourse.bass as bass  # noqa: E402
import concourse.mybir as mybir  # noqa: E402
import concourse.tile as tile  # noqa: E402
from concourse import bacc  # noqa: E402
from concourse.bass_utils import run_bass_kernel_spmd  # noqa: E402

# Problem dims (hardcoded per spec)
B, F, D = 1024, 32, 64
H = 128  # hidden per layer
NCORES = 8
BC = B // NCORES       # 128 batch per core
GB = 8                 # batch elems per group
NG = BC // GB          # 16 groups
FREE = GB * D          # 512 moving free dim
NL = 3                 # layers
NPAIR = F * (F + 1) // 2        # 528 symmetric pairs for layer 0
NC0 = (NPAIR + 127) // 128      # 5 layer-0 chunks (last one padded)

F16 = mybir.dt.float16
F32 = mybir.dt.float32

PAIR = int(os.environ.get("CIN_PAIR", "2"))        # group interleave width
ZBUFS = int(os.environ.get("CIN_ZBUFS", "4"))
PBUFS = int(os.environ.get("CIN_PBUFS", "4"))
HBUFS = int(os.environ.get("CIN_HBUFS", "6"))
XBUFS = int(os.environ.get("CIN_XBUFS", "3"))
MF = int(os.environ.get("CIN_MF", "8"))            # chunks fused per DVE mult


def build_program(repeat=1):
    nc = bacc.Bacc("TRN2", target_bir_lowering=False)

    xsa_d = nc.dram_tensor("xsa", [NG, 128, NC0, FREE], F16, kind="ExternalInput")
    xsb_d = nc.dram_tensor("xsb", [NG, 128, NC0, FREE], F16, kind="ExternalInput")
    xbase_d = nc.dram_tensor("xbase", [NG, F, FREE], F16, kind="ExternalInput")
    w0_d = nc.dram_tensor("w0", [128, NC0, 128], F16, kind="ExternalInput")
    w1_d = nc.dram_tensor("w1", [128, F, 128], F16, kind="ExternalInput")
    w2_d = nc.dram_tensor("w2", [128, F, 128], F16, kind="ExternalInput")
    b0_d = nc.dram_tensor("b0", [128, 1], F32, kind="ExternalInput")
    b1_d = nc.dram_tensor("b1", [128, 1], F32, kind="ExternalInput")
    b2_d = nc.dram_tensor("b2", [128, 1], F32, kind="ExternalInput")
    out_d = nc.dram_tensor("outy", [128, NL, NG, GB], F32, kind="ExternalOutput")

    with tile.TileContext(nc) as tc:
        with (
            tc.tile_pool(name="singles", bufs=1) as singles,
            tc.tile_pool(name="x0b", bufs=XBUFS) as x0b_pool,
            tc.tile_pool(name="sym", bufs=2) as sym_pool,
            tc.tile_pool(name="ppool", bufs=PBUFS) as p_pool,
            tc.tile_pool(name="hpool", bufs=HBUFS) as h_pool,
            tc.tile_pool(name="zpool", bufs=ZBUFS, space="PSUM") as z_pool,
        ):
            w0_sb = singles.tile([128, NC0, 128], F16)
            w1_sb = singles.tile([128, F, 128], F16)
            w2_sb = singles.tile([128, F, 128], F16)
            b0_sb = singles.tile([128, 1], F32)
            b1_sb = singles.tile([128, 1], F32)
            b2_sb = singles.tile([128, 1], F32)
            outstage = singles.tile([128, NL, NG, GB], F32)
            sum_scratch = singles.tile([128, D], F16)

            # weight/bias DMAs are emitted from inside the first prepare()
            # (after the first group's layer-0 operands) so the single SP DMA
            # ring delivers what the pipeline needs first
            def emit_w0():
                nc.sync.dma_start(out=w0_sb[:], in_=w0_d[:])
                nc.sync.dma_start(out=b0_sb[:], in_=b0_d[:])
                nc.sync.dma_start(out=b1_sb[:], in_=b1_d[:])
                nc.sync.dma_start(out=b2_sb[:], in_=b2_d[:])

            def emit_w12():
                nc.sync.dma_start(out=w1_sb[:], in_=w1_d[:])
                nc.sync.dma_start(out=w2_sb[:], in_=w2_d[:])

            w_views = [w0_sb, w1_sb, w2_sb]
            b_views = [b0_sb, b1_sb, b2_sb]

            def prep_xu(g, split=False, after_c0=None):
                """Layer-0 pair-gathered operand DMAs for group g. split=True
                interleaves per-chunk sa/sb transfers (first group only) so
                the very first multiply starts as early as possible."""
                sa_t = sym_pool.tile([128, NC0, FREE], F16, tag="sa")
                sb_t = sym_pool.tile([128, NC0, FREE], F16, tag="sb")
                if split:
                    for c in range(NC0):
                        nc.sync.dma_start(out=sa_t[:, c:c + 1],
                                          in_=xsa_d[g, :, c:c + 1])
                        nc.sync.dma_start(out=sb_t[:, c:c + 1],
                                          in_=xsb_d[g, :, c:c + 1])
                        if c == 0 and after_c0 is not None:
                            after_c0()
                else:
                    nc.sync.dma_start(out=sa_t[:], in_=xsa_d[g])
                    nc.sync.dma_start(out=sb_t[:], in_=xsb_d[g])
                return sa_t, sb_t

            def prep_x0b(g, after_q1=None):
                """x0 broadcast replication (layers 1-2 operand B), split in
                MF-chunk quarters so the first layer-1 multiply only waits
                for quarter 0. All DMAs ride the SP ring: ACT's sequencer is
                needed for the relu + d-sum stream."""
                x0b_t = x0b_pool.tile([128, F, FREE], F16, tag="x0b")
                for q0 in range(0, F, MF):
                    q1 = min(q0 + MF, F)
                    nc.sync.dma_start(
                        out=x0b_t[:, q0:q1],
                        in_=xbase_d[g, q0:q1].partition_broadcast(128),
                    )
                    if q0 == 0 and after_q1 is not None:
                        after_q1()
                return x0b_t

            def prepare(g):
                sa_t, sb_t = prep_xu(g)
                x0b_t = prep_x0b(g)
                return sa_t, x0b_t, sb_t

            pending_sums = []

            def flush_sums():
                while pending_sums:
                    pending_sums.pop(0)()

            def layer(g, l, src_h, bcast, nchunks, eager_sums=False,
                      chunk_tt=False):
                """One CIN layer for group g; returns relu'd hidden (fp16).

                src_h: [128, FREE] tile reused across chunks via stride-0 AP
                (layers 1-2), or a [128, nchunks, FREE] per-chunk operand
                (layer 0, pair-gathered)."""
                z_t = z_pool.tile([128, FREE], F32, tag="z")
                per_chunk_a = src_h.shape[1:] != (FREE,)
                sh = src_h[:]
                chunk_rhs = {}
                step = 1 if chunk_tt else MF
                for t0 in range(0, nchunks, step):
                    bs = min(step, nchunks - t0)
                    if per_chunk_a:
                        a_op = src_h[:, t0:t0 + bs]
                    else:
                        a_op = bass.AP(
                            tensor=sh.tensor, offset=sh.offset,
                            ap=[list(sh.ap[0]), [0, bs], list(sh.ap[1])],
                        )
                    p_t = p_pool.tile([128, bs, FREE], F16, tag="p")
                    nc.vector.tensor_mul(p_t[:], a_op, bcast[:, t0:t0 + bs])
                    for i in range(bs):
                        chunk_rhs[t0 + i] = p_t[:, i]
                for c in range(nchunks):
                    nc.tensor.matmul(
                        z_t[:],
                        w_views[l][:, c],
                        chunk_rhs[c],
                        start=(c == 0),
                        stop=(c == nchunks - 1),
                    )
                # Single ReLU keeps the h handoff to the next layer's DVE
                # multiply fast; the d-sums run later on ACT (Copy+accum per
                # batch elem), deferred one layer so every ReLU sits at the
                # front of ACT's strict-FIFO queue.
                h_t = h_pool.tile([128, FREE], F16, tag="h")
                nc.scalar.activation(
                    h_t[:], z_t[:], mybir.ActivationFunctionType.Relu,
                    bias=b_views[l][:],
                )
                flush_sums()

                def emit_sums(h_t=h_t, l=l, g=g):
                    for j in range(GB):
                        nc.scalar.activation(
                            sum_scratch[:],
                            h_t[:, j * D:(j + 1) * D],
                            mybir.ActivationFunctionType.Copy,
                            accum_out=outstage[:, l, g, j:j + 1],
                        )
                if eager_sums:
                    # final layer of the sweep: DVE is idle by now and ACT
                    # still has the previous layers' sums queued -- one DVE
                    # reduce per group shortens the tail
                    nc.vector.reduce_sum(
                        out=outstage[:, l, g],
                        in_=h_t.rearrange("p (b d) -> p b d", b=GB),
                        axis=mybir.AxisListType.X,
                    )
                else:
                    pending_sums.append(emit_sums)
                return h_t

            # process groups in interleaved batches of PAIR, to hide the
            # serial mult->matmul->relu dependency at layer boundaries
            npairs = NG // PAIR
            for _rep in range(repeat):
                for t in range(npairs):
                    gs = [PAIR * t + j for j in range(PAIR)]
                    if _rep == 0 and t == 0:
                        # startup DMA-queue order: g0 layer-0 operands, w0,
                        # g0 x0b (g1's layer-0 operands slipped in after the
                        # first quarter so its layer 0 can fill DVE gaps),
                        # then w1/w2 and the rest
                        xu1 = []
                        xu0 = prep_xu(gs[0], split=True, after_c0=emit_w0)
                        x0b0 = prep_x0b(
                            gs[0], after_q1=lambda: xu1.append(prep_xu(gs[1])))
                        emit_w12()
                        x0b1 = prep_x0b(gs[1])
                        preps = [(xu0[0], x0b0, xu0[1]),
                                 (xu1[0][0], x0b1, xu1[0][1])]
                        preps += [prepare(g) for g in gs[2:]]
                    else:
                        preps = [prepare(g) for g in gs]
                    last = (t == npairs - 1)
                    first = (_rep == 0 and t == 0)
                    hs = [layer(g, 0, p[0], p[2], NC0,
                                chunk_tt=(first and i == 0))
                          for i, (g, p) in enumerate(zip(gs, preps))]
                    hs = [layer(g, 1, h, p[1], F)
                          for g, h, p in zip(gs, hs, preps)]
                    for g, h, p in zip(gs, hs, preps):
                        layer(g, 2, h, p[1], F, eager_sums=last)
                    if t >= 2:
                        # ship finished pairs' outputs early; keeps the tail
                        # to the last pair's sums plus one small DMA
                        gdone = PAIR * (t - 2)
                        nc.sync.dma_start(
                            out=out_d[:, :, gdone:gdone + PAIR],
                            in_=outstage[:, :, gdone:gdone + PAIR])

                flush_sums()
                nc.sync.dma_start(
                    out=out_d[:, :, PAIR * (npairs - 2):],
                    in_=outstage[:, :, PAIR * (npairs - 2):])

    nc.finalize()
    return nc


def _sym_maps():
    """Pair order for the symmetric layer 0: r = c*128 + p -> (a(r), b(r))."""
    amap = np.zeros(NC0 * 128, dtype=np.int64)
    bmap = np.zeros(NC0 * 128, dtype=np.int64)
    r = 0
    for a in range(F):
        for b in range(a, F):
            amap[r], bmap[r] = a, b
            r += 1
    assert r == NPAIR
    return amap, bmap


def host_prep(x, W0, b0, W1, b1, W2, b2):
    """Build per-core input maps (numpy only)."""
    x = np.asarray(x, dtype=np.float32)
    assert x.shape == (B, F, D), x.shape
    xh = x.astype(np.float16)

    amap, bmap = _sym_maps()

    # layer-0 weights: symmetrize, double off-diagonal, pack pairs (a<=b)
    Wr0 = np.asarray(W0, dtype=np.float32).reshape(H, F, F)      # (o, h, m)
    S = 0.5 * (Wr0 + Wr0.transpose(0, 2, 1))
    Wp = S[:, amap, bmap]                                        # (o, 640)
    Wp[:, amap != bmap] *= 2.0
    Wp[:, NPAIR:] = 0.0
    w0l = np.ascontiguousarray(
        Wp.reshape(H, NC0, 128).transpose(2, 1, 0)).astype(np.float16)

    Wr1 = np.asarray(W1, dtype=np.float32).reshape(H, H, F)      # (o, h, m)
    w1l = np.ascontiguousarray(Wr1.transpose(1, 2, 0)).astype(np.float16)
    Wr2 = np.asarray(W2, dtype=np.float32).reshape(H, H, F)
    w2l = np.ascontiguousarray(Wr2.transpose(1, 2, 0)).astype(np.float16)

    b0c = np.asarray(b0, dtype=np.float32).reshape(128, 1)
    b1c = np.asarray(b1, dtype=np.float32).reshape(128, 1)
    b2c = np.asarray(b2, dtype=np.float32).reshape(128, 1)

    am2 = amap.reshape(NC0, 128)
    bm2 = bmap.reshape(NC0, 128)

    in_maps = []
    for i in range(NCORES):
        s = xh[i * BC:(i + 1) * BC].reshape(NG, GB, F, D)        # (g, b, m, d)
        base = np.ascontiguousarray(s.transpose(0, 2, 1, 3)).reshape(NG, F, FREE)
        # pair-gathered layer-0 operands: [NG, 128, NC0, FREE]
        xsa = np.ascontiguousarray(base[:, am2].transpose(0, 2, 1, 3))
        xsb = np.ascontiguousarray(base[:, bm2].transpose(0, 2, 1, 3))
        in_maps.append({
            "xsa": xsa,
            "xsb": xsb,
            "xbase": np.ascontiguousarray(base),
            "w0": w0l, "w1": w1l, "w2": w2l,
            "b0": b0c, "b1": b1c, "b2": b2c,
        })
    return in_maps


_NC_CACHE = {}


def _get_nc():
    if "nc" not in _NC_CACHE:
        _NC_CACHE["nc"] = build_program()
    return _NC_CACHE["nc"]


def kernel(x, W0, b0, W1, b1, W2, b2, _trace=False):
    in_maps = host_prep(x, W0, b0, W1, b1, W2, b2)
    nc = _get_nc()
    res = run_bass_kernel_spmd(nc, in_maps, list(range(NCORES)), trace=_trace)
    outs = []
    for i in range(NCORES):
        o = np.asarray(res.results[i]["outy"], dtype=np.float32)  # (128, 3, 16, 8)
        outs.append(o.transpose(2, 3, 1, 0).reshape(BC, NL * 128))
    full = np.concatenate(outs, axis=0).astype(np.float32)
    if _trace:
        return full, res
    return full


# revision 30
# speedup vs baseline: 1.5046x; 1.5046x over previous
"""Trainium2 Bass kernel for nn_CIN: 3-layer Compressed Interaction Network.

Reference computation (per layer l, with x0 = x):
    z = einsum('bhd,bmd,ohm->bod', h, x0, W_l.reshape(o, h, m)) + b_l
    h = relu(z)
Output: concat over layers of sum_d(h)  -> (B, 384)

Strategy: data-parallel over batch across 8 cores (128 b/core). Per core,
batch is processed in 16 groups of 8 (free dim f = (b_l, d) = 512).
Each layer is a chain of matmul accumulations over chunks c of 128
(h, m)-pairs:
    P_c[p, f] = A_c[p, f] * B_c[p, f]               (DVE fp16 multiply, 2x)
    z[o, f]  += W_c^T @ P_c                         (PE fp16, fp32 PSUM)
Layers 1-2 chunk as (h all 128, m = c): A_c = h (reused via a stride-0 AP),
B_c = x0 row m broadcast to 128 partitions by HWDGE DMA (the x0 broadcast
is shared by both layers). Layer 0 exploits the x (x) x symmetry: only the
528 pairs (a <= b) are materialized (5 chunks, host-symmetrized W0 with
off-diagonal weights doubled); both operands come pair-gathered from HBM.
ReLU runs on ACT; the d-sums come from ACT Copy+accum_out ops off the
critical path. Groups are processed in interleaved pairs to hide the
serial multiply -> matmul -> relu chain at layer boundaries.

Measured on 8 axon-tunneled trn2 cores: ~330 us end-to-end, rel err
~5e-4 vs the fp32 reference (fp16 operands, fp32 PSUM accumulation).
"""

import os
import sys

import numpy as np

for _p in ("/opt/trn_rl_repo", "/root/.axon_site/_ro/trn_rl_repo"):
    if os.path.isdir(_p) and _p not in sys.path:
        sys.path.append(_p)

import concourse.bass as bass  # noqa: E402
import concourse.mybir as mybir  # noqa: E402
import concourse.tile as tile  # noqa: E402
from concourse import bacc  # noqa: E402
from concourse.bass_utils import run_bass_kernel_spmd  # noqa: E402

# Problem dims (hardcoded per spec)
B, F, D = 1024, 32, 64
H = 128  # hidden per layer
NCORES = 8
BC = B // NCORES       # 128 batch per core
GB = 8                 # batch elems per group
NG = BC // GB          # 16 groups
FREE = GB * D          # 512 moving free dim
NL = 3                 # layers
NPAIR = F * (F + 1) // 2        # 528 symmetric pairs for layer 0
NC0 = (NPAIR + 127) // 128      # 5 layer-0 chunks (last one padded)

F16 = mybir.dt.float16
F32 = mybir.dt.float32

PAIR = int(os.environ.get("CIN_PAIR", "2"))        # group interleave width
ZBUFS = int(os.environ.get("CIN_ZBUFS", "4"))
PBUFS = int(os.environ.get("CIN_PBUFS", "4"))
HBUFS = int(os.environ.get("CIN_HBUFS", "6"))
XBUFS = int(os.environ.get("CIN_XBUFS", "3"))
MF = int(os.environ.get("CIN_MF", "8"))            # chunks fused per DVE mult


def build_program(repeat=1):
    nc = bacc.Bacc("TRN2", target_bir_lowering=False)

    xsa_d = nc.dram_tensor("xsa", [NG, 128, NC0, FREE], F16, kind="ExternalInput")
    xsb_d = nc.dram_tensor("xsb", [NG, 128, NC0, FREE], F16, kind="ExternalInput")
    xbase_d = nc.dram_tensor("xbase", [NG, F, FREE], F16, kind="ExternalInput")
    w0_d = nc.dram_tensor("w0", [128, NC0, 128], F16, kind="ExternalInput")
    w1_d = nc.dram_tensor("w1", [128, F, 128], F16, kind="ExternalInput")
    w2_d = nc.dram_tensor("w2", [128, F, 128], F16, kind="ExternalInput")
    b0_d = nc.dram_tensor("b0", [128, 1], F32, kind="ExternalInput")
    b1_d = nc.dram_tensor("b1", [128, 1], F32, kind="ExternalInput")
    b2_d = nc.dram_tensor("b2", [128, 1], F32, kind="ExternalInput")
    out_d = nc.dram_tensor("outy", [128, NL, NG, GB], F32, kind="ExternalOutput")

    with tile.TileContext(nc) as tc:
        with (
            tc.tile_pool(name="singles", bufs=1) as singles,
            tc.tile_pool(name="x0b", bufs=XBUFS) as x0b_pool,
            tc.tile_pool(name="sym", bufs=2) as sym_pool,
            tc.tile_pool(name="ppool", bufs=PBUFS) as p_pool,
            tc.tile_pool(name="hpool", bufs=HBUFS) as h_pool,
            tc.tile_pool(name="zpool", bufs=ZBUFS, space="PSUM") as z_pool,
        ):
            w0_sb = singles.tile([128, NC0, 128], F16)
            w1_sb = singles.tile([128, F, 128], F16)
            w2_sb = singles.tile([128, F, 128], F16)
            b0_sb = singles.tile([128, 1], F32)
            b1_sb = singles.tile([128, 1], F32)
            b2_sb = singles.tile([128, 1], F32)
            outstage = singles.tile([128, NL, NG, GB], F32)
            sum_scratch = singles.tile([128, D], F16)

            # weight/bias DMAs are emitted from inside the first prepare()
            # (after the first group's layer-0 operands) so the single SP DMA
            # ring delivers what the pipeline needs first
            def emit_w0():
                nc.sync.dma_start(out=w0_sb[:], in_=w0_d[:])
                nc.sync.dma_start(out=b0_sb[:], in_=b0_d[:])
                nc.sync.dma_start(out=b1_sb[:], in_=b1_d[:])
                nc.sync.dma_start(out=b2_sb[:], in_=b2_d[:])

            def emit_w12():
                nc.sync.dma_start(out=w1_sb[:], in_=w1_d[:])
                nc.sync.dma_start(out=w2_sb[:], in_=w2_d[:])

            w_views = [w0_sb, w1_sb, w2_sb]
            b_views = [b0_sb, b1_sb, b2_sb]

            def prep_xu(g, split=False, after_c0=None):
                """Layer-0 pair-gathered operand DMAs for group g. split=True
                interleaves per-chunk sa/sb transfers (first group only) so
                the very first multiply starts as early as possible."""
                sa_t = sym_pool.tile([128, NC0, FREE], F16, tag="sa")
                sb_t = sym_pool.tile([128, NC0, FREE], F16, tag="sb")
                if split:
                    for c0, c1 in ((0, 2), (2, NC0)):
                        nc.sync.dma_start(out=sa_t[:, c0:c1],
                                          in_=xsa_d[g, :, c0:c1])
                        nc.sync.dma_start(out=sb_t[:, c0:c1],
                                          in_=xsb_d[g, :, c0:c1])
                        if c0 == 0 and after_c0 is not None:
                            after_c0()
                else:
                    nc.sync.dma_start(out=sa_t[:], in_=xsa_d[g])
                    nc.sync.dma_start(out=sb_t[:], in_=xsb_d[g])
                return sa_t, sb_t

            def prep_x0b(g, after_q1=None):
                """x0 broadcast replication (layers 1-2 operand B), split in
                MF-chunk quarters so the first layer-1 multiply only waits
                for quarter 0. All DMAs ride the SP ring: ACT's sequencer is
                needed for the relu + d-sum stream."""
                x0b_t = x0b_pool.tile([128, F, FREE], F16, tag="x0b")
                for q0 in range(0, F, MF):
                    q1 = min(q0 + MF, F)
                    nc.sync.dma_start(
                        out=x0b_t[:, q0:q1],
                        in_=xbase_d[g, q0:q1].partition_broadcast(128),
                    )
                    if q0 == 0 and after_q1 is not None:
                        after_q1()
                return x0b_t

            def prepare(g):
                sa_t, sb_t = prep_xu(g)
                x0b_t = prep_x0b(g)
                return sa_t, x0b_t, sb_t

            pending_sums = []

            def flush_sums():
                while pending_sums:
                    pending_sums.pop(0)()

            def layer(g, l, src_h, bcast, nchunks, eager_sums=False,
                      chunk_tt=False):
                """One CIN layer for group g; returns relu'd hidden (fp16).

                src_h: [128, FREE] tile reused across chunks via stride-0 AP
                (layers 1-2), or a [128, nchunks, FREE] per-chunk operand
                (layer 0, pair-gathered)."""
                z_t = z_pool.tile([128, FREE], F32, tag="z")
                per_chunk_a = src_h.shape[1:] != (FREE,)
                sh = src_h[:]
                chunk_rhs = {}
                step = 2 if chunk_tt else MF
                for t0 in range(0, nchunks, step):
                    bs = min(step, nchunks - t0)
                    if per_chunk_a:
                        a_op = src_h[:, t0:t0 + bs]
                    else:
                        a_op = bass.AP(
                            tensor=sh.tensor, offset=sh.offset,
                            ap=[list(sh.ap[0]), [0, bs], list(sh.ap[1])],
                        )
                    p_t = p_pool.tile([128, bs, FREE], F16, tag="p")
                    nc.vector.tensor_mul(p_t[:], a_op, bcast[:, t0:t0 + bs])
                    for i in range(bs):
                        chunk_rhs[t0 + i] = p_t[:, i]
                for c in range(nchunks):
                    nc.tensor.matmul(
                        z_t[:],
                        w_views[l][:, c],
                        chunk_rhs[c],
                        start=(c == 0),
                        stop=(c == nchunks - 1),
                    )
                # Single ReLU keeps the h handoff to the next layer's DVE
                # multiply fast; the d-sums run later on ACT (Copy+accum per
                # batch elem), deferred one layer so every ReLU sits at the
                # front of ACT's strict-FIFO queue.
                h_t = h_pool.tile([128, FREE], F16, tag="h")
                nc.scalar.activation(
                    h_t[:], z_t[:], mybir.ActivationFunctionType.Relu,
                    bias=b_views[l][:],
                )
                flush_sums()

                def emit_sums(h_t=h_t, l=l, g=g):
                    for j in range(GB):
                        nc.scalar.activation(
                            sum_scratch[:],
                            h_t[:, j * D:(j + 1) * D],
                            mybir.ActivationFunctionType.Copy,
                            accum_out=outstage[:, l, g, j:j + 1],
                        )
                if eager_sums:
                    # final layer of the sweep: DVE is idle by now and ACT
                    # still has the previous layers' sums queued -- one DVE
                    # reduce per group shortens the tail
                    nc.vector.reduce_sum(
                        out=outstage[:, l, g],
                        in_=h_t.rearrange("p (b d) -> p b d", b=GB),
                        axis=mybir.AxisListType.X,
                    )
                else:
                    pending_sums.append(emit_sums)
                return h_t

            # process groups in interleaved batches of PAIR, to hide the
            # serial mult->matmul->relu dependency at layer boundaries
            npairs = NG // PAIR
            for _rep in range(repeat):
                for t in range(npairs):
                    gs = [PAIR * t + j for j in range(PAIR)]
                    if _rep == 0 and t == 0:
                        # startup DMA-queue order: g0 layer-0 operands, w0,
                        # g0 x0b (g1's layer-0 operands slipped in after the
                        # first quarter so its layer 0 can fill DVE gaps),
                        # then w1/w2 and the rest
                        xu1 = []
                        xu0 = prep_xu(gs[0], split=True, after_c0=emit_w0)
                        x0b0 = prep_x0b(
                            gs[0], after_q1=lambda: xu1.append(prep_xu(gs[1])))
                        emit_w12()
                        x0b1 = prep_x0b(gs[1])
                        preps = [(xu0[0], x0b0, xu0[1]),
                                 (xu1[0][0], x0b1, xu1[0][1])]
                        preps += [prepare(g) for g in gs[2:]]
                    else:
                        preps = [prepare(g) for g in gs]
                    last = (t == npairs - 1)
                    first = (_rep == 0 and t == 0)
                    hs = [layer(g, 0, p[0], p[2], NC0,
                                chunk_tt=(first and i == 0))
                          for i, (g, p) in enumerate(zip(gs, preps))]
                    hs = [layer(g, 1, h, p[1], F)
                          for g, h, p in zip(gs, hs, preps)]
                    for g, h, p in zip(gs, hs, preps):
                        layer(g, 2, h, p[1], F, eager_sums=last)
                    if t >= 2:
                        # ship finished pairs' outputs early; keeps the tail
                        # to the last pair's sums plus one small DMA
                        gdone = PAIR * (t - 2)
                        nc.sync.dma_start(
                            out=out_d[:, :, gdone:gdone + PAIR],
                            in_=outstage[:, :, gdone:gdone + PAIR])

                flush_sums()
                nc.sync.dma_start(
                    out=out_d[:, :, PAIR * (npairs - 2):],
                    in_=outstage[:, :, PAIR * (npairs - 2):])

    nc.finalize()
    return nc


def _sym_maps():
    """Pair order for the symmetric layer 0: r = c*128 + p -> (a(r), b(r))."""
    amap = np.zeros(NC0 * 128, dtype=np.int64)
    bmap = np.zeros(NC0 * 128, dtype=np.int64)
    r = 0
    for a in range(F):
        for b in range(a, F):
            amap[r], bmap[r] = a, b
            r += 1
    assert r == NPAIR
    return amap, bmap


def host_prep(x, W0, b0, W1, b1, W2, b2):
    """Build per-core input maps (numpy only)."""
    x = np.asarray(x, dtype=np.float32)
    assert x.shape == (B, F, D), x.shape
    xh = x.astype(np.float16)

    amap, bmap = _sym_maps()

    # layer-0 weights: symmetrize, double off-diagonal, pack pairs (a<=b)
    Wr0 = np.asarray(W0, dtype=np.float32).reshape(H, F, F)      # (o, h, m)
    S = 0.5 * (Wr0 + Wr0.transpose(0, 2, 1))
    Wp = S[:, amap, bmap]                                        # (o, 640)
    Wp[:, amap != bmap] *= 2.0
    Wp[:, NPAIR:] = 0.0
    w0l = np.ascontiguousarray(
        Wp.reshape(H, NC0, 128).transpose(2, 1, 0)).astype(np.float16)

    Wr1 = np.asarray(W1, dtype=np.float32).reshape(H, H, F)      # (o, h, m)
    w1l = np.ascontiguousarray(Wr1.transpose(1, 2, 0)).astype(np.float16)
    Wr2 = np.asarray(W2, dtype=np.float32).reshape(H, H, F)
    w2l = np.ascontiguousarray(Wr2.transpose(1, 2, 0)).astype(np.float16)

    b0c = np.asarray(b0, dtype=np.float32).reshape(128, 1)
    b1c = np.asarray(b1, dtype=np.float32).reshape(128, 1)
    b2c = np.asarray(b2, dtype=np.float32).reshape(128, 1)

    am2 = amap.reshape(NC0, 128)
    bm2 = bmap.reshape(NC0, 128)

    in_maps = []
    for i in range(NCORES):
        s = xh[i * BC:(i + 1) * BC].reshape(NG, GB, F, D)        # (g, b, m, d)
        base = np.ascontiguousarray(s.transpose(0, 2, 1, 3)).reshape(NG, F, FREE)
        # pair-gathered layer-0 operands: [NG, 128, NC0, FREE]
        xsa = np.ascontiguousarray(base[:, am2].transpose(0, 2, 1, 3))
        xsb = np.ascontiguousarray(base[:, bm2].transpose(0, 2, 1, 3))
        in_maps.append({
            "xsa": xsa,
            "xsb": xsb,
            "xbase": np.ascontiguousarray(base),
            "w0": w0l, "w1": w1l, "w2": w2l,
            "b0": b0c, "b1": b1c, "b2": b2c,
        })
    return in_maps


_NC_CACHE = {}


def _get_nc():
    if "nc" not in _NC_CACHE:
        _NC_CACHE["nc"] = build_program()
    return _NC_CACHE["nc"]


def kernel(x, W0, b0, W1, b1, W2, b2, _trace=False):
    in_maps = host_prep(x, W0, b0, W1, b1, W2, b2)
    nc = _get_nc()
    res = run_bass_kernel_spmd(nc, in_maps, list(range(NCORES)), trace=_trace)
    outs = []
    for i in range(NCORES):
        o = np.asarray(res.results[i]["outy"], dtype=np.float32)  # (128, 3, 16, 8)
        outs.append(o.transpose(2, 3, 1, 0).reshape(BC, NL * 128))
    full = np.concatenate(outs, axis=0).astype(np.float32)
    if _trace:
        return full, res
    return full
